# revision 24
# baseline (speedup 1.0000x reference)
"""Trainium2 Bass kernel for the noisy quantized KWS LSTM.

Strategy (data-parallel, memory-regime):
  - Shard batch B=1024 across 8 NeuronCores (128 per core).
  - Per-timestep weight noise (jax threefry, fold_in(key(42), t)) is
    reproduced exactly on host CPU; effective weights W_eff[t] =
    quant(w) + noise[t] stream from HBM in bf16, chunked along T with a
    partition-major host layout so each DMA moves few, large
    descriptors.
  - State kept transposed ([hidden, batch]); per-step gates.T [1024,128]
    accumulate in PSUM as two [128, 512] groups G_j = [i_j f_j g_j o_j]
    (j = hidden chunk) so the serial recurrence chain pipelines at chunk
    granularity.
  - tanh(g) = 2*sigmoid(2g)-1 with the x2 folded into the g-gate weights
    on host, so one sigmoid covers each group; g's clip-at-0 rides a
    fused (i*g max 0) + f*c scalar_tensor_tensor op (valid since i>=0).
  - Quantization (round-half-even to 1/256, 1/128 grids) via the
    magic-constant trick on the DVE, all tiles bf16 for 2x/4x DVE modes.
"""

import os
import sys

os.environ.setdefault("MYCRO_LOCAL_CACHE", "1")
sys.path.insert(0, "/opt/trn_rl_repo")

from contextlib import ExitStack

import ml_dtypes
import numpy as np

# ---------------- problem constants (hardcoded per contract) ----------------
T = 256
B = 1024
I_DIM = 40
H = 256
O_DIM = 12
G4 = 4 * H  # 1024
N_CORES = 8
BSH = B // N_CORES  # 128
NOISE_LEVEL = 0.1

XCH = 8   # timesteps per WX dma chunk
HCH = 4   # timesteps per WH dma chunk

DEBUG = False  # dump step-0 intermediates to extra outputs

C256 = 32768.0  # 2^15: ulp = 1/256 on [2^15, 2^16)
C128 = 65536.0  # 2^16: ulp = 1/128 on [2^16, 2^17)


def _quant_np(x, bits, sign):
    scale = np.float32(2.0 ** (bits - 1) if sign else 2.0**bits)
    y = np.clip(x.astype(np.float32), np.float32(0.0), np.float32(1.0))
    return (np.round(y * scale) / scale).astype(np.float32)


def _prepare_host(inputs, w_ih, w_hh, b_ih, b_hh, out_w, out_b):
    import jax

    cpu = jax.devices("cpu")[0]

    qx = _quant_np(inputs, 8, True)  # [T, B, I] on 1/128 grid in [0,1]
    qw_ih_t = _quant_np(w_ih.T, 8, True)  # [I, 4H]
    qw_hh_t = _quant_np(w_hh.T, 8, True)  # [H, 4H]
    qb = _quant_np(b_ih, 8, True) + _quant_np(b_hh, 8, True)  # [4H]
    wmax_ih = np.float32(np.max(w_ih))
    wmax_hh = np.float32(np.max(w_hh))

    # gate column permutation: reference order [i f g o] along 4H ->
    # chunk groups [i_j f_j g_j o_j] (j = hidden 128-chunk); g chunks
    # scaled x2 (tanh(x) = 2*sigmoid(2x) - 1).
    perm = np.concatenate(
        [
            np.arange(0, 128),      # i0
            np.arange(256, 384),    # f0
            np.arange(512, 640),    # g0
            np.arange(768, 896),    # o0
            np.arange(128, 256),    # i1
            np.arange(384, 512),    # f1
            np.arange(640, 768),    # g1
            np.arange(896, 1024),   # o1
        ]
    )
    gscale = np.ones((G4,), dtype=np.float32)
    gscale[256:384] = 2.0   # g0 position after perm
    gscale[768:896] = 2.0   # g1 position after perm

    # partition-major layouts so per-chunk DMAs are few large descriptors
    WX = np.zeros((128, T, G4), dtype=ml_dtypes.bfloat16)
    WH = np.empty((128, T, 2 * G4), dtype=ml_dtypes.bfloat16)

    import jax.numpy as jnp

    CHUNK = min(32, T)

    def gen_chunk(t0):
        with jax.default_device(cpu):
            nkey = jax.random.key(42)
            ts_ = jnp.arange(t0, t0 + CHUNK)
            keys = jax.vmap(lambda t: jax.random.fold_in(nkey, t))(ts_)
            k12 = jax.vmap(jax.random.split)(keys)  # [CHUNK, 2]
            n_ih = jax.vmap(
                lambda k: jax.random.normal(k, (I_DIM, G4), dtype=jnp.float32)
            )(k12[:, 0])
            n_hh = jax.vmap(
                lambda k: jax.random.normal(k, (H, G4), dtype=jnp.float32)
            )(k12[:, 1])
        return np.asarray(n_ih), np.asarray(n_hh)

    bias_row = (qb[perm] * gscale).astype(ml_dtypes.bfloat16)
    for t0 in range(0, T, CHUNK):
        n_ih, n_hh = gen_chunk(t0)
        n_ih = (n_ih * wmax_ih) * np.float32(NOISE_LEVEL)
        n_hh = (n_hh * wmax_hh) * np.float32(NOISE_LEVEL)
        wx_eff = (qw_ih_t[None] + n_ih)[:, :, perm] * gscale  # [CHUNK, I, 4H]
        wh_eff = (qw_hh_t[None] + n_hh)[:, :, perm] * gscale  # [CHUNK, H, 4H]
        WX[:I_DIM, t0 : t0 + CHUNK, :] = np.transpose(
            wx_eff.astype(ml_dtypes.bfloat16), (1, 0, 2)
        )
        WX[I_DIM, t0 : t0 + CHUNK, :] = bias_row[None]
        whb = wh_eff.astype(ml_dtypes.bfloat16)
        WH[:, t0 : t0 + CHUNK, :G4] = np.transpose(whb[:, :128, :], (1, 0, 2))
        WH[:, t0 : t0 + CHUNK, G4:] = np.transpose(whb[:, 128:, :], (1, 0, 2))

    # per-core resident x.T with ones row: [41, T*BSH]
    XTs = []
    for c in range(N_CORES):
        xs = qx[:, c * BSH : (c + 1) * BSH, :]  # [T, BSH, I]
        xt = np.zeros((128, T * BSH), dtype=ml_dtypes.bfloat16)
        xt[:I_DIM, :] = np.transpose(xs, (2, 0, 1)).reshape(I_DIM, T * BSH)
        xt[I_DIM, :] = np.float32(1.0)
        XTs.append(xt)

    OW = np.empty((128, 2 * O_DIM), dtype=ml_dtypes.bfloat16)
    OW[:, :O_DIM] = out_w[:, :128].T
    OW[:, O_DIM:] = out_w[:, 128:].T
    OB = out_b.astype(np.float32).reshape(O_DIM, 1)
    return WX, WH, XTs, OW, OB



_DVE_OPS_CACHE = {}


def _get_custom_ops():
    """Register the kernel's fused DVE ops in dve_ops' registry (idempotent).

    KWS_IG:  out = in0 * (in1 * s0 - 1)            (i_q * (2*sig(2g) - 1))
    KWS_CPQ: out = min(relu(in0) + in1 + s0 - s0, 1)   (c update + quant128)
    KWS_HQ:  out = ((in0 + s0 - s0) * in1 + s1) - s1   (quant256(o)*th, quant128)
    """
    if _DVE_OPS_CACHE:
        return _DVE_OPS_CACHE
    import numpy as np
    from concourse import dve_ops
    from concourse.dve_spec import (
        C0, C1, C2, One, Spec, Src0, Src1, _has_src1, lower, minn, relu,
    )
    from concourse.dve_uop import DveOpSpec

    B1, B2, B3 = 0.008223789081935651, -0.39286107309956864, 0.1463125399188885
    defs = {
        "KWS_FC": (
            ((Src0 + C0) - C0) * Src1,
            lambda in0, in1, s0, s1, imm2: ((in0 + s0) - s0) * in1,
        ),
        "KWS_IG2": (
            ((Src0 + C0) - C0) * ((Src1 * C1) - One),
            lambda in0, in1, s0, s1, imm2: ((in0 + s0) - s0) * (in1 * s1 - 1.0),
        ),
        "KWS_TANH": (
            Src0 * ((((Src0 * C0) + C1) * Src0 + C2) * Src0 + One),
            lambda in0, in1, s0, s1, imm2: in0
            * (((in0 * s0 + s1) * in0 + imm2) * in0 + 1.0),
        ),
        "KWS_CPQ": (
            minn(((relu(Src0) + Src1) + C0) - C0, One),
            lambda in0, in1, s0, s1, imm2: np.minimum(
                (np.maximum(in0, 0.0) + in1 + s0) - s0, 1.0
            ),
        ),
        "KWS_HQ": (
            ((((Src0 + C0) - C0) * Src1) + C1) - C1,
            lambda in0, in1, s0, s1, imm2: (((in0 + s0) - s0) * in1 + s1) - s1,
        ),
    }
    for name, (body, ref) in defs.items():
        if name in dve_ops._SUB_OPCODE_FOR_NAME:
            _DVE_OPS_CACHE[name] = next(o for o in dve_ops.OPS if o.name == name)
            continue
        row = dve_ops._CUSTOM_DVE_ROW_BASE + len(dve_ops.OPS)
        assert row < 0x20, "custom-DVE opcode rows exhausted"
        spec = Spec(body=body, reference=ref)
        shas = {}
        for ver in ("v3", "v4"):
            try:
                uops = lower(spec, ver=ver)
            except Exception:
                continue
            shas[ver] = DveOpSpec(
                name=name, opcode=row, uops=uops, rd1_en=_has_src1(spec)
            ).sha(ver)
        op = dve_ops.DveOp(name, spec, subdim=False, uops_sha=shas)
        dve_ops.OPS.append(op)
        dve_ops._SUB_OPCODE_FOR_NAME[name] = row
        dve_ops.CUSTOM_DVE_SPECS[name] = spec
        _DVE_OPS_CACHE[name] = op
    return _DVE_OPS_CACHE


def _build_bass():
    import concourse.bass as bass
    import concourse.tile as tile
    from concourse import bacc, mybir

    AF = mybir.ActivationFunctionType
    AO = mybir.AluOpType
    f32 = mybir.dt.float32
    bf16 = mybir.dt.bfloat16

    nc = bacc.Bacc("TRN2", target_bir_lowering=False, debug=False)

    WX_d = nc.dram_tensor("WX", [128, T, G4], bf16, kind="ExternalInput")
    WH_d = nc.dram_tensor("WH", [128, T, 2 * G4], bf16, kind="ExternalInput")
    XT_d = nc.dram_tensor("XT", [128, T * BSH], bf16, kind="ExternalInput")
    OW_d = nc.dram_tensor("OW", [128, 2 * O_DIM], bf16, kind="ExternalInput")
    OB_d = nc.dram_tensor("OB", [O_DIM, 1], f32, kind="ExternalInput")
    OUT_d = nc.dram_tensor("OUT", [O_DIM, BSH], f32, kind="ExternalOutput")
    if DEBUG:
        DBG_SG = [
            nc.dram_tensor(f"DBG_SG{j}", [128, 512], f32, kind="ExternalOutput")
            for j in range(2)
        ]
        DBG_H = [
            nc.dram_tensor(f"DBG_H{j}", [128, BSH], f32, kind="ExternalOutput")
            for j in range(2)
        ]
        DBG_C = [
            nc.dram_tensor(f"DBG_C{j}", [128, BSH], f32, kind="ExternalOutput")
            for j in range(2)
        ]

    CUST = _get_custom_ops()

    with tile.TileContext(nc) as tc, ExitStack() as ctx:
        singles = ctx.enter_context(tc.tile_pool(name="singles", bufs=1))
        wh_pool = ctx.enter_context(tc.tile_pool(name="whp", bufs=3))
        wx_pool = ctx.enter_context(tc.tile_pool(name="wxp", bufs=2))
        st_pool = ctx.enter_context(tc.tile_pool(name="st", bufs=2))
        work = ctx.enter_context(tc.tile_pool(name="work", bufs=2))
        pp = ctx.enter_context(tc.tile_pool(name="pp", bufs=2, space="PSUM"))
        pp1 = ctx.enter_context(tc.tile_pool(name="pp1", bufs=1, space="PSUM"))

        xt = singles.tile([128, T * BSH], bf16)
        nc.sync.dma_start(out=xt, in_=XT_d[:, :])
        ow = singles.tile([128, 2 * O_DIM], bf16)
        nc.sync.dma_start(out=ow, in_=OW_d[:, :])
        ob = singles.tile([O_DIM, 1], f32)
        nc.sync.dma_start(out=ob, in_=OB_d[:, :])

        # recurrent state per hidden chunk j: h_j, c_j are [128, 128]
        hs, cs = [], []
        for j in range(2):
            h = st_pool.tile([128, BSH], bf16, tag=f"h{j}")
            nc.vector.memset(h, 0.0)
            c = st_pool.tile([128, BSH], bf16, tag=f"c{j}")
            nc.vector.memset(c, 0.0)
            hs.append(h)
            cs.append(c)

        wx_chunks = [None, None]
        wh_chunks = [None, None, None]
        pss = [None, None]

        def fetch_wx(ci):
            wx = wx_pool.tile([128, XCH * G4], bf16, tag="wx")
            nc.sync.dma_start(
                out=wx, in_=WX_d[:, ci * XCH : (ci + 1) * XCH, :]
            )
            wx_chunks[ci % 2] = wx

        def fetch_wh(ci):
            wh = wh_pool.tile([128, HCH * 2 * G4], bf16, tag="wh")
            nc.sync.dma_start(
                out=wh, in_=WH_d[:, ci * HCH : (ci + 1) * HCH, :]
            )
            wh_chunks[ci % 3] = wh

        sgw_next = [xt]  # rhs for the HAM-filler matmuls (prev step's sg0)

        fetch_wx(0)
        fetch_wx(1)
        fetch_wh(0)
        fetch_wh(1)
        fetch_wh(2)

        for t in range(T):
            # prefetch the next weight chunk at each chunk boundary: all
            # readers of the buffer being recycled were emitted in earlier
            # iterations, so the pool's WAR edge is complete
            if t % XCH == 2 and (t // XCH + 1) * XCH < T:
                fetch_wx(t // XCH + 1)
            if t > 0 and t % HCH == 0 and (t // HCH + 2) * HCH < T:
                fetch_wh(t // HCH + 2)

            wx = wx_chunks[(t // XCH) % 2][
                :, (t % XCH) * G4 : (t % XCH + 1) * G4
            ]
            whc = wh_chunks[(t // HCH) % 3]
            wh = whc[:, (t % HCH) * 2 * G4 : (t % HCH + 1) * 2 * G4]
            xts = xt[:, t * BSH : (t + 1) * BSH]

            # All matmuls for one psum chunk are contiguous (x start, then
            # hh k0/k1 accumulates). Interleaving chunks is NOT safe: each
            # start=True marks its whole 2KB psum bank pending-zero, so a
            # later accumulate into an earlier-started chunk zeroes it.
            # psum per group: [i f g] tile (sigmoid fires after 9 MMs) and
            # a SHARED o-tile for both groups (o path runs late anyway).
            def chunk_mms(psg, col, mm, stop_k1=True):
                nc.tensor.matmul(
                    psg[:, col : col + 128],
                    wx[:, mm * 128 : (mm + 1) * 128],
                    xts,
                    start=True,
                    stop=False,
                )
                for k in range(2):
                    nc.tensor.matmul(
                        psg[:, col : col + 128],
                        wh[:, k * G4 + mm * 128 : k * G4 + (mm + 1) * 128],
                        hs[k],
                        start=False,
                        stop=(k == 1),
                    )

            ps = []
            for g in range(2):
                psg = pp.tile([128, 384], f32, tag=f"ps{g}")
                ps.append(psg)
                for m in range(3):  # i, f, g chunks
                    chunk_mms(psg, m * 128, g * 4 + m)
            pso = pp.tile([128, 256], f32, tag="pso")
            for g in range(2):  # o chunks, both groups in one tile
                chunk_mms(pso, g * 128, g * 4 + 3)

            sgw = sgw_next[0]
            # Keep the PE's HAM activity window busy through the pointwise
            # tail so the clock gate stays open (2.4 GHz): dummy matmuls
            # anchored on this step's first sigmoid output fill the gap.
            warm = pp1.tile([24, BSH], f32, tag="warm")
            for _ in range(14):
                nc.tensor.matmul(warm, ow[:, 0:24], sgw[:, 0:BSH], start=True, stop=True)

            # ACT queue: sig(ifg0), sig(ifg1), sig(o01)
            sg = []
            for j in range(2):
                s = work.tile([128, 384], bf16, tag=f"sg{j}")
                nc.scalar.activation(s, ps[j], AF.Sigmoid)
                sg.append(s)
            sgw_next[0] = sg[0]
            so = work.tile([128, 256], bf16, tag="so")
            nc.scalar.activation(so, pso, AF.Sigmoid)

            # DVE queue: per group: fused quant(f)*c, fused quant(i)*g,
            # fused c-update, cubic tanh; tails: fused h quant
            ths = []
            for j in range(2):
                f_ = work.tile([128, 128], bf16, tag=f"fc{j}")
                nc.vector._custom_dve(
                    CUST["KWS_FC"], out=f_, in0=sg[j][:, 128:256],
                    in1=cs[j], s0=C256,
                )
                i_ = work.tile([128, 128], bf16, tag=f"ig{j}")
                nc.vector._custom_dve(
                    CUST["KWS_IG2"], out=i_, in0=sg[j][:, 0:128],
                    in1=sg[j][:, 256:384], s0=C256, s1=2.0,
                )
                c = st_pool.tile([128, BSH], bf16, tag=f"c{j}")
                nc.vector._custom_dve(
                    CUST["KWS_CPQ"], out=c, in0=i_, in1=f_, s0=C128,
                )
                cs[j] = c
                th = work.tile([128, 128], bf16, tag=f"th{j}")
                nc.vector._custom_dve(
                    CUST["KWS_TANH"], out=th, in0=c,
                    s0=0.1463125399188885, s1=-0.39286107309956864,
                    imm2=0.008223789081935651,
                )
                ths.append(th)
                # h right after tanh: h0 lands 2 ops earlier so the next
                # step's k0 matmuls overlap G1's remaining chain
                h = st_pool.tile([128, BSH], bf16, tag=f"h{j}")
                nc.vector._custom_dve(
                    CUST["KWS_HQ"], out=h, in0=so[:, j * 128 : (j + 1) * 128],
                    in1=ths[j], s0=C256, s1=C128,
                )
                hs[j] = h

        pf = pp1.tile([O_DIM, BSH], f32, tag="pf")
        nc.tensor.matmul(pf, ow[:, 0:O_DIM], hs[0], start=True, stop=False)
        nc.tensor.matmul(pf, ow[:, O_DIM:], hs[1], start=False, stop=True)
        so = work.tile([O_DIM, BSH], f32, tag="sgout")
        nc.scalar.activation(so, pf, AF.Sigmoid, bias=ob[:, :])
        oq = work.tile([O_DIM, BSH], f32, tag="oq")
        nc.vector.tensor_scalar(oq, so, C256, C256, AO.add, AO.subtract)
        nc.sync.dma_start(out=OUT_d[:, :], in_=oq)

    return nc


_RUN_KW = {}  # test.py can inject trace=True etc.


def kernel(inputs, w_ih, w_hh, b_ih, b_hh, out_w, out_b):
    from concourse.bass_utils import run_bass_kernel_spmd

    WX, WH, XTs, OW, OB = _prepare_host(
        inputs, w_ih, w_hh, b_ih, b_hh, out_w, out_b
    )
    nc = _build_bass()
    if not nc.is_finalized():
        nc.finalize()
    in_maps = [
        {"WX": WX, "WH": WH, "XT": XTs[c], "OW": OW, "OB": OB}
        for c in range(N_CORES)
    ]
    res = run_bass_kernel_spmd(nc, in_maps, core_ids=list(range(N_CORES)), **_RUN_KW)
    kernel.last_results = res
    out = np.concatenate([r["OUT"].T for r in res.results], axis=0)  # [B, O]
    return out.astype(np.float32)
